# revision 21
# baseline (speedup 1.0000x reference)
"""Masked dot-product attention (B=16, Q=K=2048, D=64) on 8 Trainium2 cores.

Strategy
--------
softmax(QK^T/8 + mask) @ V with per-batch valid_lens. Work is sharded at
(batch, 512-wide q-block) granularity: 64 units whose cost is
nk(b) = ceil(valid_len[b]/128) k-tiles. Units are sorted by nk descending and
dealt into 8 slots x 8 cores, so every core runs the *same* static program
(slot j processes NK_j k-tiles) while the host packs each core's own data.

Main loop per k-tile pair (all matmul inputs fp16; PSUM accumulates fp32):
  PE : S^T[128k, 512q] for TWO k-tiles at once via row-group packing
       (tile A on partitions 0-63, tile B on 64-127; the host duplicates
       the d=64 rows of Q^T/K^T into both halves), each k-tile writing its
       own PSUM bank of ps[128, 1024].
  ACT: P = exp(S^T / 8)   (one ACTIVATE over both banks)
  PE : O^T_aug[65, 512q] += matmul(lhsT=V_aug-tile[128,65], rhs=P-half)
V_aug = [V | 1] has rows >= valid_len zeroed by the host: the zeroed rows
implement the attention mask exactly (invalid keys contribute nothing to the
numerator or the ones-column denominator), so no masking or row-max pass is
needed on device; exp() without max-subtraction is safe since scores ~ N(0,1).
O-matmuls are emitted one pair behind the S-matmuls so the PE queue never
head-of-line blocks waiting for exp.

Epilogue per unit (overlapped with later units): copy numerator + denominator
out of PSUM fast (recycling the accumulator bank), DMA the denominator row
into a gather tile; reciprocals run batched (DVE iterative divide is 8
cycles/element and FD-serial, so batching units across partitions amortizes
it; the final batch uses ScalarE ln->exp(-x) because ACT is idle at the
tail), broadcast across the 64 d-partitions via a DRAM-bounce DMA
(partition-step-0 read), multiply, DMA O^T out. The host transposes
O^T -> O while unsharding.
"""

import sys

if "/opt/trn_rl_repo" not in sys.path:
    sys.path.insert(0, "/opt/trn_rl_repo")

import numpy as np

import concourse.bass as bass
import concourse.mybir as mybir
import concourse.tile as tile
from concourse import bacc
from concourse.bass_utils import run_bass_kernel_spmd

B, Q, KLEN, D = 16, 2048, 2048, 64
QB = 512                      # q-block width per work unit
NCORES = 8
NSLOTS = (B * (Q // QB)) // NCORES   # 8 slots per core
KT = 128                      # k-tile height
F32 = mybir.dt.float32
F16 = mybir.dt.float16
NPF16 = np.float16

LAST_RESULTS = None           # BassKernelResults of the most recent run

_cache: dict = {}


def _schedule(valid_lens):
    """Static work schedule from valid_lens (host-known at call time)."""
    nk = [max(1, -(-int(v) // KT)) for v in valid_lens]
    units = [(b, qb) for b in range(B) for qb in range(Q // QB)]
    units.sort(key=lambda u: (-nk[u[0]], u))
    slots_nk = [nk[units[NCORES * j][0]] for j in range(NSLOTS)]
    assign = [[units[NCORES * j + c] for j in range(NSLOTS)] for c in range(NCORES)]
    offs = np.concatenate([[0], np.cumsum(slots_nk)]).tolist()
    return nk, slots_nk, offs, assign


def _build(slots_nk, offs):
    """Build + compile the single SPMD program for the given slot profile."""
    # small slots first: compute starts almost immediately, and the big
    # late slots' long windows hide every earlier unit's division pipeline
    order = sorted(range(NSLOTS), key=lambda j: slots_nk[j])
    xw = [QB + w * KT + w * 65 for w in slots_nk]
    xoffs = np.concatenate([[0], np.cumsum([xw[j] for j in order])]).tolist()
    # reciprocal batches over unit positions (in processing order); the two
    # final (largest) units divide individually so only the very last unit's
    # chain sits on the tail
    rbatches = [[0, 1, 2], [3, 4, 5], [6], [7]]

    nc = bacc.Bacc()
    data_d = nc.dram_tensor("data", [2 * D, xoffs[-1]], F16,
                            kind="ExternalInput").ap()
    out_d = nc.dram_tensor("out", [NSLOTS, D, QB], F32, kind="ExternalOutput").ap()

    with tile.TileContext(nc) as tc:
        with (
            tc.tile_pool(name="spool", bufs=3) as spool,
            tc.tile_pool(name="ppool", bufs=4) as ppool,
            tc.tile_pool(name="epool", bufs=2) as epool,
            tc.tile_pool(name="gpool", bufs=1) as gpool,
            tc.tile_pool(name="opool", bufs=2) as opool,
            tc.tile_pool(name="dpool", bufs=2, space="DRAM") as dpool,
            tc.tile_pool(name="psum_s", bufs=3, space="PSUM") as psum_s,
            tc.tile_pool(name="psum_o", bufs=2, space="PSUM") as psum_o,
        ):
            dn_tiles = {}
            for bi, ub in enumerate(rbatches):
                dn_tiles[bi] = gpool.tile([len(ub), QB], F32, name=f"dn{bi}",
                                          tag=f"dn{bi}")
            o_tiles = []

            def emit_division(bi, ub, dn):
                # all division-pipeline DMAs ride the idle GpSimd SWDGE queue
                # so they never head-of-line block the SP input-DMA FIFO
                r_sb = epool.tile([len(ub), QB], F32, tag="r")
                nc.vector.reciprocal(r_sb, dn)
                scratch = dpool.tile([len(ub), QB], F32, tag="scr")
                nc.gpsimd.dma_start(out=scratch, in_=r_sb)
                rb_sb = epool.tile([D, len(ub), QB], F32, tag="rb")
                bcast_src = bass.AP(
                    tensor=scratch.tensor,
                    offset=scratch.offset,
                    ap=[[0, D]] + [list(a) for a in scratch.ap],
                )
                nc.gpsimd.dma_start(out=rb_sb, in_=bcast_src)
                for ui, jj in enumerate(ub):
                    oo_sb = opool.tile([D, QB], F32, tag="oo")
                    nc.vector.tensor_mul(oo_sb, o_tiles[jj],
                                         rb_sb[:, ui, :])
                    nc.gpsimd.dma_start(out=out_d[order[jj]], in_=oo_sb)

            # flat schedule of (unit position, k-tile group) so the
            # S->exp->O software pipeline flows across slot boundaries
            # without flushing
            slot_ctx = {}

            def open_slot(jidx):
                j = order[jidx]
                w = slots_nk[j]
                x_sb = spool.tile([2 * D, xw[j]], F16, tag="x")
                nc.sync.dma_start(
                    out=x_sb, in_=data_d[:, xoffs[jidx]:xoffs[jidx] + xw[j]])
                po = psum_o.tile([65, QB], F32, tag="po")
                slot_ctx[jidx] = (x_sb, po, w)

            def close_slot(jidx):
                # free the accumulator bank fast, stage the denominator row
                # into this unit's gather batch, fire division when complete
                _, po, _ = slot_ctx[jidx]
                oa_sb = gpool.tile([D, QB], F32, name=f"oa{jidx}",
                                   tag=f"oa{jidx}")
                nc.vector.tensor_copy(oa_sb, po[0:64, :])
                o_tiles.append(oa_sb)
                dcp = epool.tile([1, QB], F32, tag="dcp")
                nc.vector.tensor_copy(dcp, po[64:65, :])
                bi = next(i for i, ub in enumerate(rbatches) if jidx in ub)
                ri = rbatches[bi].index(jidx)
                if len(rbatches[bi]) == 1:
                    # singleton batch: reciprocate straight from the copy
                    emit_division(bi, rbatches[bi], dcp)
                else:
                    nc.gpsimd.dma_start(out=dn_tiles[bi][ri:ri + 1, :],
                                        in_=dcp)
                    if rbatches[bi][-1] == jidx:
                        emit_division(bi, rbatches[bi], dn_tiles[bi])

            sched = []
            for jidx, j in enumerate(order):
                w = slots_nk[j]
                for g in range((w + 1) // 2):
                    sched.append((jidx, g))

            pending = None      # (jidx, [(ki, ph, p_sb)...], closes_slot)
            for jidx, g in sched:
                if g == 0:
                    open_slot(jidx)
                x_sb, po, w = slot_ctx[jidx]
                qt_sb = x_sb[:, 0:QB]
                kt_sb = x_sb[:, QB:QB + w * KT]
                va_sb = x_sb[:, QB + w * KT:].rearrange(
                    "p (w c) -> p w c", c=65)
                hs = [h for h in (0, 1) if g * 2 + h < w]
                ww = len(hs) * QB
                ps = psum_s.tile([128, 2 * QB], F32, tag="ps")
                for h in hs:
                    ki = g * 2 + h
                    rg = h * D      # row group: 0-63 or 64-127
                    nc.tensor.matmul(
                        ps[:, h * QB:(h + 1) * QB],
                        lhsT=kt_sb[rg:rg + D, ki * KT:(ki + 1) * KT],
                        rhs=qt_sb[rg:rg + D, :],
                        start=True, stop=True,
                        tile_position=(rg, 0),
                    )
                if pending is not None:
                    pj, items, closes = pending
                    _, ppo, pw = slot_ctx[pj]
                    pva = slot_ctx[pj][0][:, QB + pw * KT:].rearrange(
                        "p (w c) -> p w c", c=65)
                    for ki, ph, p_prev in items:
                        nc.tensor.matmul(
                            ppo,
                            lhsT=pva[:, ki, :],
                            rhs=p_prev[:, ph * QB:(ph + 1) * QB],
                            start=(ki == 0), stop=(ki == pw - 1),
                        )
                    if closes:
                        close_slot(pj)
                p_sb = ppool.tile([128, 2 * QB], F16, tag="p")
                nc.scalar.activation(
                    p_sb[:, :ww], ps[:, :ww],
                    mybir.ActivationFunctionType.Exp, scale=0.125,
                )
                pending = (jidx, [(g * 2 + h, h, p_sb) for h in hs],
                           g == (w + 1) // 2 - 1)
            pj, items, closes = pending
            _, ppo, pw = slot_ctx[pj]
            pva = slot_ctx[pj][0][:, QB + pw * KT:].rearrange(
                "p (w c) -> p w c", c=65)
            for ki, ph, p_prev in items:
                nc.tensor.matmul(
                    ppo,
                    lhsT=pva[:, ki, :],
                    rhs=p_prev[:, ph * QB:(ph + 1) * QB],
                    start=(ki == 0), stop=(ki == pw - 1),
                )
            close_slot(pj)

    nc.compile()
    return nc


def _pack(queries, keys, values, valid_lens, slots_nk, offs, assign):
    order = sorted(range(NSLOTS), key=lambda j: slots_nk[j])
    xw = [QB + w * KT + w * 65 for w in slots_nk]
    tot = sum(xw)
    data = np.zeros((NCORES, 2 * D, tot), NPF16)
    for c in range(NCORES):
        x0 = 0
        for j in order:
            b, qb = assign[c][j]
            w = slots_nk[j]
            vl = int(valid_lens[b])
            blk = data[c, :, x0:x0 + xw[j]]
            qt = queries[b, qb * QB:(qb + 1) * QB, :].T        # [D, QB]
            blk[:D, 0:QB] = qt
            blk[D:, 0:QB] = qt
            ktr = keys[b, :w * KT, :].T                        # [D, w*KT]
            blk[:D, QB:QB + w * KT] = ktr
            blk[D:, QB:QB + w * KT] = ktr
            vv = np.zeros((w * KT, 65), np.float32)
            vv[:vl, :D] = values[b, :vl, :]
            vv[:vl, D] = 1.0
            # [128 partitions, w, 65] flattened on the free axis
            blk[:, QB + w * KT:] = (
                vv.reshape(w, KT, 65).transpose(1, 0, 2).reshape(KT, w * 65))
            x0 += xw[j]
    return [{"data": data[c]} for c in range(NCORES)]


def kernel(queries, keys, values, valid_lens):
    global LAST_RESULTS
    queries = np.asarray(queries, dtype=np.float32)
    keys = np.asarray(keys, dtype=np.float32)
    values = np.asarray(values, dtype=np.float32)
    valid_lens = np.asarray(valid_lens)

    key = tuple(int(v) for v in valid_lens)
    if key not in _cache:
        nk, slots_nk, offs, assign = _schedule(valid_lens)
        nc = _build(slots_nk, offs)
        _cache[key] = (nc, slots_nk, offs, assign)
    nc, slots_nk, offs, assign = _cache[key]

    in_maps = _pack(queries, keys, values, valid_lens, slots_nk, offs, assign)
    res = run_bass_kernel_spmd(nc, in_maps, list(range(NCORES)))
    LAST_RESULTS = res

    out = np.empty((B, Q, D), np.float32)
    for c in range(NCORES):
        oc = res.results[c]["out"]          # [NSLOTS, D, QB]
        for j in range(NSLOTS):
            b, qb = assign[c][j]
            out[b, qb * QB:(qb + 1) * QB, :] = oc[j].T
    return out
